# revision 14
# baseline (speedup 1.0000x reference)
"""Bass/Trainium2 kernel for a 2-layer GAT (GATConv x2 + log_softmax) on 8 NeuronCores.

Strategy (edge/data parallel, dst-sharded):
  - Nodes sharded 8 ways by id; core c owns dst nodes [c*SH, (c+1)*SH).
  - Phase A (per core): h = feat_shard @ W1, el/er per node.  [h|el] written to
    a gather table (768B rows), AllGathered to every core.  er kept in a local
    256B-row table.
  - Phase B (L1 edges, partitioned by dst owner): per 128-edge tile,
    dma_gather h[src] rows from HBM, dma_gather er[dst] rows, build a one-hot
    B[edge, dst_slot] matrix on DVE, and aggregate messages with PE matmuls:
    psum[slot, :] += B^T @ [exp(leaky(el+er)) * h[src] | exp(...)].
    Normalize by the attention-sum column, add bias, ELU -> h2.
  - L2 node phase fused per chunk: hh|el2|er2 = h2T @ [W2|W2@al2|W2@ar2]
    (single matmul after a PE transpose), written to the L2 table + AllGather.
  - Phase C: same edge pipeline on 256B rows, then log_softmax, output shard.

Edges per core are sorted by (dst chunk of 128 slots, src window), padded to
128-edge tiles with counts uniform across cores so one SPMD program serves all
8 cores.  src gather indices are int16 with 2 overlapping windows (the base
points into the middle of the table; negative indices address below it).
No segment-max is needed: attention logits are O(1) so exp() cannot overflow.
"""

import os
import sys

import numpy as np

sys.path.insert(0, "/opt/trn_rl_repo")

# ---------------------------------------------------------------- constants
N_NODES = 100000
F_IN = 256
HID = 16
HEADS = 8
N_CLASSES = 16
NEG_SLOPE = 0.2
NC = 8                      # cores
CH = 128                    # dst nodes per chunk
BLK = 2                     # chunks per block (gather granularity)
GMAX = 8                    # max 128-edge tiles per dma_gather instruction
ROW1 = 192                  # f32 per L1 table row (h 128 | el 8 | pad)
ROW2 = 64                   # f32 per L2 table row (hh 16 | el2 1 | pad)
WIN_EDGE = 65536            # src window split (int16 gather idx)


WIN = 32768  # max rows addressable by one (non-negative) int16 gather window


def _win_bases(n_nodes, win_edge=None):
    """Positive-index windows: window w covers rows [w*WIN, min((w+1)*WIN, n))."""
    nwin = (n_nodes + WIN - 1) // WIN
    bases = [w * WIN for w in range(nwin)]
    return bases, bases


def _wrap16(v):
    # [n] -> [128, n//16] int16; idx i at [i%16, i//16], replicated over groups
    n = v.shape[0]
    assert n % 16 == 0
    a = v.reshape(n // 16, 16).T.astype(np.int16)      # [16, n//16]
    return np.ascontiguousarray(np.tile(a, (8, 1)))    # [128, n//16]


def host_prep(src, dst, n_nodes=N_NODES, nc=NC, ch=CH, win_edge=WIN_EDGE):
    """Partition/sort/pad edges; build gather index + slot arrays per core."""
    sh = n_nodes // nc
    nchunk = (sh + ch - 1) // ch
    bases, wstarts = _win_bases(n_nodes)
    nwin = len(bases)

    per_core = []
    cnt = np.zeros((nc, nchunk, nwin), dtype=np.int64)
    for c in range(nc):
        m = (dst // sh) == c
        es, ed = src[m].astype(np.int64), (dst[m] - c * sh).astype(np.int64)
        chunk = ed // ch
        bank = np.zeros_like(es)
        for w in range(1, nwin):
            bank[es >= wstarts[w]] = w
        order = np.lexsort((bank, chunk))
        es, ed, chunk, bank = es[order], ed[order], chunk[order], bank[order]
        per_core.append((es, ed))
        for k in range(nchunk):
            km = chunk == k
            for w in range(nwin):
                cnt[c, k, w] = np.count_nonzero(km & (bank == w))

    # uniform tile counts: max over cores, ceil to 128-edge tiles
    T = np.maximum(np.ceil(cnt.max(axis=0) / 128.0), 1).astype(np.int64)

    nblk = (nchunk + BLK - 1) // BLK
    # stream layout: for blk: for w: for k in blk: seg(k, w) of T[k,w] tiles
    seg_off = {}
    gath = []             # (block, window, tile_off, ntiles)
    toff = 0
    for b in range(nblk):
        ks = list(range(b * BLK, min((b + 1) * BLK, nchunk)))
        for w in range(nwin):
            g0 = toff
            for k in ks:
                seg_off[(k, w)] = toff
                toff += int(T[k, w])
            gath.append((b, w, g0, toff - g0))
    ntile = toff
    ne_pad = ntile * 128

    gidx = np.zeros((nc, ne_pad), dtype=np.int16)
    didx = np.zeros((nc, ne_pad), dtype=np.int16)
    slot = np.full((nc, ne_pad), -1, dtype=np.int32)
    for c in range(nc):
        es, ed = per_core[c]
        p = 0
        ptr = {}
        for k in range(nchunk):
            for w in range(nwin):
                n = int(cnt[c, k, w])
                ptr[(k, w)] = (p, p + n)
                p += n
        for (k, w), off in seg_off.items():
            a, bnd = ptr[(k, w)]
            n = bnd - a
            pos = off * 128
            gidx[c, pos:pos + n] = (es[a:bnd] - bases[w]).astype(np.int16)
            didx[c, pos:pos + n] = ed[a:bnd].astype(np.int16)
            slot[c, pos:pos + n] = (ed[a:bnd] % ch).astype(np.int32)

    chunk_tiles = []
    for k in range(nchunk):
        tl = []
        for w in range(nwin):
            off = seg_off[(k, w)]
            tl.extend(range(off, off + int(T[k, w])))
        chunk_tiles.append(tl)

    return dict(
        n_nodes=n_nodes, sh=sh, nchunk=nchunk, nwin=nwin, bases=bases,
        nblk=nblk, ntile=ntile, ne_pad=ne_pad, gath=gath,
        chunk_tiles=chunk_tiles, seg_off=seg_off, T=T,
        gidx=gidx, didx=didx, slot=slot,
    )


# ------------------------------------------------------------- bass program
def build_program(meta, f_in, hid, heads, n_classes):
    from contextlib import ExitStack

    import concourse.tile as tile
    from concourse import bacc, mybir

    dt = mybir.dt
    f32, bf16, i16, i32 = dt.float32, dt.bfloat16, dt.int16, dt.int32
    AF = mybir.ActivationFunctionType
    OP = mybir.AluOpType
    AX = mybir.AxisListType

    n_nodes = meta["n_nodes"]
    sh, nchunk, nwin = meta["sh"], meta["nchunk"], meta["nwin"]
    nblk, ntile, ne_pad = meta["nblk"], meta["ntile"], meta["ne_pad"]
    gath, chunk_tiles = meta["gath"], meta["chunk_tiles"]
    wbase = meta["bases"]
    hd = heads * hid
    kt = f_in // 128
    ntile_a = (sh + 127) // 128
    n_last = sh - (ntile_a - 1) * 128

    nc_ = bacc.Bacc("TRN2", target_bir_lowering=False, debug=False,
                    num_devices=NC)

    def din(name, shape, dtype):
        return nc_.dram_tensor(name, list(shape), dtype,
                               kind="ExternalInput").ap()

    featT = din("featT", [f_in, sh], f32)
    W1 = din("W1", [f_in, hd], f32)
    alrep = din("alrep", [128, hd], f32)
    arrep = din("arrep", [128, hd], f32)
    b1rep = din("b1rep", [128, hd], f32)
    W2a = din("W2a", [hd, n_classes + 2], f32)
    b2rep = din("b2rep", [128, n_classes], f32)
    gidx_d = din("gidx", [128, ne_pad // 16], i16)
    didx_d = din("didx", [128, ne_pad // 16], i16)
    slot_d = din("slot", [128, ntile], f32)
    out_d = nc_.dram_tensor("out", [sh, n_classes], f32,
                            kind="ExternalOutput").ap()

    htab = nc_.dram_tensor("htab", [n_nodes, ROW1], f32).ap()
    l2tab = nc_.dram_tensor("l2tab", [n_nodes, ROW2], f32).ap()
    hshard = nc_.dram_tensor("hshard", [sh, ROW1], f32).ap()
    l2shard = nc_.dram_tensor("l2shard", [sh, ROW2], f32).ap()
    ertab = nc_.dram_tensor("ertab", [sh, 64], f32).ap()
    er2tab = nc_.dram_tensor("er2tab", [sh, 64], f32).ap()

    replica = [list(range(NC))]

    with tile.TileContext(nc_) as tc:
        nc = tc.nc
        with ExitStack() as cctx:
            cpool = cctx.enter_context(tc.tile_pool(name="const", bufs=1))
            w1_sb = cpool.tile([128, kt * hd], f32, tag="w1")
            for k in range(kt):
                nc.sync.dma_start(w1_sb[:, k * hd:(k + 1) * hd],
                                  W1[k * 128:(k + 1) * 128, :])
            al_sb = cpool.tile([128, hd], f32, tag="al")
            nc.sync.dma_start(al_sb[:], alrep[:])
            ar_sb = cpool.tile([128, hd], f32, tag="ar")
            nc.sync.dma_start(ar_sb[:], arrep[:])
            b1_sb = cpool.tile([128, hd], f32, tag="b1")
            nc.sync.dma_start(b1_sb[:], b1rep[:])
            w2_sb = cpool.tile([hd, n_classes + 2], f32, tag="w2")
            nc.sync.dma_start(w2_sb[:], W2a[:])
            b2_sb = cpool.tile([128, n_classes], f32, tag="b2")
            nc.sync.dma_start(b2_sb[:], b2rep[:])
            iota_sb = cpool.tile([128, 128], f32, tag="iota")
            nc.gpsimd.iota(iota_sb[:], pattern=[[1, 128]], base=0,
                           channel_multiplier=0,
                           allow_small_or_imprecise_dtypes=True)
            iota_p = cpool.tile([128, 1], f32, tag="iotap")
            nc.gpsimd.iota(iota_p[:], pattern=[[0, 1]], base=0,
                           channel_multiplier=1,
                           allow_small_or_imprecise_dtypes=True)
            ident_sb = cpool.tile([128, 128], f32, tag="ident")
            nc.vector.tensor_scalar(out=ident_sb[:], in0=iota_sb[:],
                                    scalar1=iota_p[:], scalar2=None,
                                    op0=OP.is_equal)

            # ---------------- phase A ----------------
            with ExitStack() as actx:
                apool = actx.enter_context(tc.tile_pool(name="phA", bufs=3))
                apsum = actx.enter_context(
                    tc.tile_pool(name="phAps", bufs=3, space="PSUM"))
                for t in range(ntile_a):
                    m = 128 if t < ntile_a - 1 else n_last
                    ft = apool.tile([128, kt, 128], f32, tag="ft")
                    for k in range(kt):
                        nc.sync.dma_start(
                            ft[:, k, :m],
                            featT[k * 128:(k + 1) * 128,
                                  t * 128:t * 128 + m])
                    ps = apsum.tile([128, hd], f32, tag="hps")
                    for k in range(kt):
                        nc.tensor.matmul(
                            ps[:m, :], lhsT=ft[:, k, :m],
                            rhs=w1_sb[:, k * hd:(k + 1) * hd],
                            start=(k == 0), stop=(k == kt - 1))
                    row = apool.tile([128, ROW1], f32, tag="row")
                    nc.scalar.copy(row[:m, 0:hd], ps[:m, :])
                    tmp = apool.tile([128, hd], f32, tag="tmpa")
                    nc.vector.tensor_mul(tmp[:m, :], ps[:m, :], al_sb[:m, :])
                    nc.vector.tensor_reduce(
                        out=row[:m, hd:hd + heads],
                        in_=tmp[:m, :].rearrange("p (h d) -> p h d", h=heads),
                        axis=AX.X, op=OP.add)
                    ert = apool.tile([128, 64], f32, tag="ert")
                    nc.vector.tensor_mul(tmp[:m, :], ps[:m, :], ar_sb[:m, :])
                    nc.vector.tensor_reduce(
                        out=ert[:m, 0:heads],
                        in_=tmp[:m, :].rearrange("p (h d) -> p h d", h=heads),
                        axis=AX.X, op=OP.add)
                    nc.vector.memset(row[:m, hd + heads:ROW1], 0.0)
                    nc.vector.memset(ert[:m, heads:64], 0.0)
                    nc.sync.dma_start(hshard[t * 128:t * 128 + m, :],
                                      row[:m, :])
                    nc.sync.dma_start(ertab[t * 128:t * 128 + m, :],
                                      ert[:m, :])

            nc.gpsimd.collective_compute(
                "AllGather", OP.bypass, replica_groups=replica,
                ins=[hshard.opt()], outs=[htab.opt()])

            # ---------------- edge phases ----------------
            def post_chunk_l1(k, ps, mrows, ppost, pps2):
                fw, sw = hd, heads
                den = ppost.tile([128, sw], f32, tag="den")
                nc.vector.tensor_scalar_max(den[:mrows, :],
                                            ps[:mrows, fw:fw + sw], 1e-30)
                rec = ppost.tile([128, sw], f32, tag="rec")
                nc.vector.reciprocal(rec[:mrows, :], den[:mrows, :])
                h2 = ppost.tile([128, fw], f32, tag="h2")
                nc.vector.tensor_mul(
                    h2[:mrows, :].rearrange("p (s d) -> p s d", s=sw),
                    ps[:mrows, 0:fw].rearrange("p (s d) -> p s d", s=sw),
                    rec[:mrows, :].unsqueeze(2)
                    .broadcast_to([mrows, sw, fw // sw]))
                nc.vector.tensor_add(h2[:mrows, :], h2[:mrows, :],
                                     b1_sb[:mrows, :])
                mn = ppost.tile([128, fw], f32, tag="mn")
                nc.vector.tensor_scalar_min(mn[:mrows, :], h2[:mrows, :], 0.0)
                nc.scalar.activation(mn[:mrows, :], mn[:mrows, :], AF.Exp)
                nc.vector.scalar_tensor_tensor(
                    out=h2[:mrows, :], in0=h2[:mrows, :], scalar=0.0,
                    in1=mn[:mrows, :], op0=OP.max, op1=OP.add)
                nc.vector.tensor_scalar_sub(h2[:mrows, :], h2[:mrows, :], 1.0)
                # L2 node phase
                pst = pps2.tile([128, 128], f32, tag="pst")
                nc.tensor.transpose(pst[:, :mrows], h2[:mrows, :],
                                    ident_sb[:mrows, :mrows])
                h2T = ppost.tile([128, 128], f32, tag="h2T")
                nc.scalar.copy(h2T[:, :mrows], pst[:, :mrows])
                ps2 = pps2.tile([128, n_classes + 2], f32, tag="hh")
                nc.tensor.matmul(ps2[:mrows, :], lhsT=h2T[:, :mrows],
                                 rhs=w2_sb[:], start=True, stop=True)
                l2r = ppost.tile([128, ROW2], f32, tag="l2r")
                nc.scalar.copy(l2r[:mrows, 0:n_classes + 1],
                               ps2[:mrows, 0:n_classes + 1])
                nc.vector.memset(l2r[:mrows, n_classes + 1:ROW2], 0.0)
                er2r = ppost.tile([128, 64], f32, tag="er2r")
                nc.scalar.copy(er2r[:mrows, 0:1],
                               ps2[:mrows, n_classes + 1:n_classes + 2])
                nc.vector.memset(er2r[:mrows, 1:64], 0.0)
                nc.sync.dma_start(l2shard[k * CH:k * CH + mrows, :],
                                  l2r[:mrows, :])
                nc.sync.dma_start(er2tab[k * CH:k * CH + mrows, :],
                                  er2r[:mrows, :])

            def post_chunk_l2(k, ps, mrows, ppost):
                fw = n_classes
                den = ppost.tile([128, 1], f32, tag="den2")
                nc.vector.tensor_scalar_max(den[:mrows, :],
                                            ps[:mrows, fw:fw + 1], 1e-30)
                rec = ppost.tile([128, 1], f32, tag="rec2")
                nc.vector.reciprocal(rec[:mrows, :], den[:mrows, :])
                xx = ppost.tile([128, fw], f32, tag="xx")
                nc.vector.tensor_scalar(out=xx[:mrows, :],
                                        in0=ps[:mrows, 0:fw],
                                        scalar1=rec[:mrows, :], scalar2=None,
                                        op0=OP.mult)
                nc.vector.tensor_add(xx[:mrows, :], xx[:mrows, :],
                                     b2_sb[:mrows, :])
                rmax = ppost.tile([128, 1], f32, tag="rmax")
                nc.vector.tensor_reduce(out=rmax[:mrows, :], in_=xx[:mrows, :],
                                        axis=AX.X, op=OP.max)
                nc.vector.tensor_scalar(out=xx[:mrows, :], in0=xx[:mrows, :],
                                        scalar1=rmax[:mrows, :], scalar2=None,
                                        op0=OP.subtract)
                exs = ppost.tile([128, fw], f32, tag="exs")
                ssum = ppost.tile([128, 1], f32, tag="ssum")
                nc.scalar.activation(exs[:mrows, :], xx[:mrows, :], AF.Exp,
                                     accum_out=ssum[:mrows, :])
                lss = ppost.tile([128, 1], f32, tag="lss")
                nc.scalar.activation(lss[:mrows, :], ssum[:mrows, :], AF.Ln)
                nc.vector.tensor_scalar(out=xx[:mrows, :], in0=xx[:mrows, :],
                                        scalar1=lss[:mrows, :], scalar2=None,
                                        op0=OP.subtract)
                nc.sync.dma_start(out_d[k * CH:k * CH + mrows, :],
                                  xx[:mrows, :])

            def edge_phase(layer):
                if layer == 1:
                    tab, ertb, rw, fw, sw = htab, ertab, ROW1, hd, heads
                else:
                    tab, ertb, rw, fw, sw = l2tab, er2tab, ROW2, n_classes, 1
                nw = fw + sw

                gblocks = {}
                for (b, w, g0, nt) in gath:
                    gblocks.setdefault(b, []).append((w, g0, nt))

                with ExitStack() as ectx:
                    pool = ectx.enter_context(
                        tc.tile_pool(name=f"edge{layer}", bufs=2))
                    pps = ectx.enter_context(
                        tc.tile_pool(name=f"eps{layer}", bufs=3,
                                     space="PSUM"))
                    ppost = ectx.enter_context(
                        tc.tile_pool(name=f"post{layer}", bufs=2))
                    pps2 = ectx.enter_context(
                        tc.tile_pool(name=f"ep2{layer}", bufs=2,
                                     space="PSUM"))
                    for b in range(nblk):
                        segs = gblocks[b]
                        t0 = segs[0][1]
                        tb = sum(s[2] for s in segs)
                        gt = pool.tile([128, tb, rw], f32, tag="gt")
                        for (w, g0, nt) in segs:
                            if nt == 0:
                                continue
                            ii = pool.tile([128, nt * 8], i16, tag="gi")
                            nc.sync.dma_start(
                                ii[:], gidx_d[:, g0 * 8:(g0 + nt) * 8])
                            wend = min(wbase[w] + 32768, n_nodes)
                            for s0 in range(0, nt, GMAX):
                                sn = min(GMAX, nt - s0)
                                nc.gpsimd.dma_gather(
                                    out_ap=gt[:, g0 - t0 + s0:
                                              g0 - t0 + s0 + sn, :],
                                    in_ap=tab[wbase[w]:wend, :],
                                    idxs_ap=ii[:, s0 * 8:(s0 + sn) * 8],
                                    num_idxs=sn * 128,
                                    num_idxs_reg=sn * 128, elem_size=rw,
                                    queue_num=0)
                        di = pool.tile([128, tb * 8], i16, tag="di")
                        nc.sync.dma_start(di[:],
                                          didx_d[:, t0 * 8:(t0 + tb) * 8])
                        erg = pool.tile([128, tb, 64], f32, tag="erg")
                        for s0 in range(0, tb, GMAX):
                            sn = min(GMAX, tb - s0)
                            nc.gpsimd.dma_gather(
                                out_ap=erg[:, s0:s0 + sn, :], in_ap=ertb[:],
                                idxs_ap=di[:, s0 * 8:(s0 + sn) * 8],
                                num_idxs=sn * 128, num_idxs_reg=sn * 128,
                                elem_size=64, queue_num=0)
                        sl = pool.tile([128, tb], f32, tag="sl")
                        nc.sync.dma_start(sl[:], slot_d[:, t0:t0 + tb])
                        B = pool.tile([128, tb, 128], bf16, tag="B")
                        nc.vector.tensor_tensor(
                            out=B[:],
                            in0=iota_sb[:].unsqueeze(1)
                            .broadcast_to([128, tb, 128]),
                            in1=sl[:].unsqueeze(2)
                            .broadcast_to([128, tb, 128]),
                            op=OP.is_equal)
                        ex = pool.tile([128, tb, sw], f32, tag="ex")
                        nc.vector.tensor_add(ex[:], gt[:, :, fw:fw + sw],
                                             erg[:, :, 0:sw])
                        nc.vector.scalar_tensor_tensor(
                            out=ex[:], in0=ex[:], scalar=NEG_SLOPE,
                            in1=ex[:], op0=OP.mult, op1=OP.max)
                        nc.scalar.activation(ex[:], ex[:], AF.Exp)
                        comb = pool.tile([128, tb, nw], bf16, tag="comb")
                        nc.scalar.copy(comb[:, :, fw:fw + sw], ex[:])
                        nc.vector.tensor_mul(
                            comb[:, :, 0:fw].rearrange(
                                "p t (s d) -> p t s d", s=sw),
                            gt[:, :, 0:fw].rearrange(
                                "p t (s d) -> p t s d", s=sw),
                            ex[:].unsqueeze(3)
                            .broadcast_to([128, tb, sw, fw // sw]))
                        for k in range(b * BLK, min((b + 1) * BLK, nchunk)):
                            tl = chunk_tiles[k]
                            ps = pps.tile([128, nw], f32, tag="agg")
                            for j, t in enumerate(tl):
                                nc.tensor.matmul(
                                    ps[:], lhsT=B[:, t - t0, :],
                                    rhs=comb[:, t - t0, :],
                                    start=(j == 0), stop=(j == len(tl) - 1))
                            mrows = min(CH, sh - k * CH)
                            if layer == 1:
                                post_chunk_l1(k, ps, mrows, ppost, pps2)
                            else:
                                post_chunk_l2(k, ps, mrows, ppost)

            edge_phase(1)
            nc.gpsimd.collective_compute(
                "AllGather", OP.bypass, replica_groups=replica,
                ins=[l2shard.opt()], outs=[l2tab.opt()])
            edge_phase(2)

    nc_.compile()
    return nc_


# ------------------------------------------------------------------ driver
def make_in_maps(meta, feat, W1, al1, ar1, b1, W2, al2, ar2, b2):
    sh = meta["sh"]
    alrep = np.tile(al1.reshape(1, -1), (128, 1)).astype(np.float32)
    arrep = np.tile(ar1.reshape(1, -1), (128, 1)).astype(np.float32)
    b1rep = np.tile(b1.reshape(1, -1), (128, 1)).astype(np.float32)
    W2a = np.concatenate(
        [W2, W2 @ al2.reshape(-1, 1), W2 @ ar2.reshape(-1, 1)],
        axis=1).astype(np.float32)
    b2rep = np.tile(b2.reshape(1, -1), (128, 1)).astype(np.float32)
    in_maps = []
    for c in range(NC):
        in_maps.append({
            "featT": np.ascontiguousarray(feat[c * sh:(c + 1) * sh, :].T),
            "W1": W1, "alrep": alrep, "arrep": arrep, "b1rep": b1rep,
            "W2a": W2a, "b2rep": b2rep,
            "gidx": _wrap16(meta["gidx"][c]),
            "didx": _wrap16(meta["didx"][c]),
            "slot": np.ascontiguousarray(
                meta["slot"][c].reshape(-1, 128).T).astype(np.float32),
        })
    return in_maps


class Runner:
    """Builds the SPMD program once; exposes a repeatable timed executor."""

    def __init__(self, meta, f_in):
        self.meta = meta
        self.nc = build_program(meta, f_in, HID, HEADS, N_CLASSES)
        self._fn = None

    def _lower(self):
        import jax
        import numpy as _np
        from jax.sharding import Mesh, PartitionSpec
        from jax.experimental.shard_map import shard_map
        from concourse import mybir
        from concourse.bass2jax import _bass_exec_p, install_neuronx_cc_hook

        install_neuronx_cc_hook()
        nc = self.nc
        in_names, out_names, out_avals, zero_outs = [], [], [], []
        partition_name = (nc.partition_id_tensor.name
                          if nc.partition_id_tensor else None)
        for alloc in nc.m.functions[0].allocations:
            if not isinstance(alloc, mybir.MemoryLocationSet):
                continue
            name = alloc.memorylocations[0].name
            if alloc.kind == "ExternalInput":
                if name != partition_name:
                    in_names.append(name)
            elif alloc.kind == "ExternalOutput":
                shape = tuple(alloc.tensor_shape)
                dtype = mybir.dt.np(alloc.dtype)
                out_names.append(name)
                out_avals.append(jax.core.ShapedArray(shape, dtype))
                zero_outs.append(_np.zeros(shape, dtype))
        n_params = len(in_names)
        n_outs = len(out_avals)
        all_in_names = list(in_names) + list(out_names)
        if partition_name is not None:
            all_in_names.append(partition_name)

        def _body(*args):
            operands = list(args)
            if partition_name is not None:
                from concourse.bass2jax import partition_id_tensor
                operands.append(partition_id_tensor())
            outs = _bass_exec_p.bind(
                *operands, out_avals=tuple(out_avals),
                in_names=tuple(all_in_names), out_names=tuple(out_names),
                lowering_input_output_aliases=(),
                sim_require_finite=True, sim_require_nnan=True, nc=nc)
            return tuple(outs)

        devices = jax.devices()[:NC]
        mesh = Mesh(_np.asarray(devices), ("core",))
        in_specs = (PartitionSpec("core"),) * (n_params + n_outs)
        out_specs = (PartitionSpec("core"),) * n_outs
        self._fn = jax.jit(
            shard_map(_body, mesh=mesh, in_specs=in_specs,
                      out_specs=out_specs, check_rep=False),
            keep_unused=True)
        self._in_names = in_names
        self._out_names = out_names
        self._out_avals = out_avals
        self._zero_outs = zero_outs
        self._mesh = mesh
        self._in_specs = in_specs

    def prepare(self, in_maps):
        import jax
        import numpy as _np
        from jax.sharding import NamedSharding, PartitionSpec
        if self._fn is None:
            self._lower()
        concat_in = [
            _np.concatenate([in_maps[c][name] for c in range(NC)], axis=0)
            for name in self._in_names]
        concat_zeros = [
            _np.zeros((NC * z.shape[0], *z.shape[1:]), z.dtype)
            for z in self._zero_outs]
        shd = NamedSharding(self._mesh, PartitionSpec("core"))
        self._args = [jax.device_put(a, shd) for a in concat_in + concat_zeros]
        jax.block_until_ready(self._args)

    def run(self):
        import jax
        out = self._fn(*self._args)
        out = jax.block_until_ready(out)
        import numpy as _np
        res = _np.asarray(out[self._out_names.index("out")])
        sh = self._out_avals[self._out_names.index("out")].shape
        return res.reshape(NC, *sh).reshape(NC * sh[0], *sh[1:])


_RUNNER = None


def get_runner(feat, src, dst):
    global _RUNNER
    n, f_in = feat.shape
    meta = host_prep(np.asarray(src, np.int32), np.asarray(dst, np.int32),
                     n_nodes=n)
    _RUNNER = Runner(meta, f_in)
    return _RUNNER


def kernel(feat, src, dst, W1, al1, ar1, b1, W2, al2, ar2, b2):
    feat = np.asarray(feat, dtype=np.float32)
    src = np.asarray(src, dtype=np.int32)
    dst = np.asarray(dst, dtype=np.int32)
    args = [np.asarray(x, np.float32)
            for x in (W1, al1, ar1, b1, W2, al2, ar2, b2)]
    r = _RUNNER if _RUNNER is not None else get_runner(feat, src, dst)
    in_maps = make_in_maps(r.meta, feat, *args)
    r.prepare(in_maps)
    return r.run()


kernel.last_exec_time_ns = None


# revision 15
# speedup vs baseline: 32.2663x; 32.2663x over previous
"""Bass/Trainium2 kernel for a 2-layer GAT (GATConv x2 + log_softmax) on 8 NeuronCores.

Strategy (edge/data parallel, dst-sharded):
  - Nodes sharded 8 ways by id; core c owns dst nodes [c*SH, (c+1)*SH).
  - Phase A (per core): h = feat_shard @ W1, el/er per node.  [h|el] written to
    a gather table (768B rows), AllGathered to every core.  er kept in a local
    256B-row table.
  - Phase B (L1 edges, partitioned by dst owner): per 128-edge tile,
    dma_gather h[src] rows from HBM, dma_gather er[dst] rows, build a one-hot
    B[edge, dst_slot] matrix on DVE, and aggregate messages with PE matmuls:
    psum[slot, :] += B^T @ [exp(leaky(el+er)) * h[src] | exp(...)].
    Normalize by the attention-sum column, add bias, ELU -> h2.
  - L2 node phase fused per chunk: hh|el2|er2 = h2T @ [W2|W2@al2|W2@ar2]
    (single matmul after a PE transpose), written to the L2 table + AllGather.
  - Phase C: same edge pipeline on 256B rows, then log_softmax, output shard.

Edges per core are sorted by (dst chunk of 128 slots, src window), padded to
128-edge tiles with counts uniform across cores so one SPMD program serves all
8 cores.  src gather indices are int16 with 2 overlapping windows (the base
points into the middle of the table; negative indices address below it).
No segment-max is needed: attention logits are O(1) so exp() cannot overflow.
"""

import os
import sys

import numpy as np

sys.path.insert(0, "/opt/trn_rl_repo")

# ---------------------------------------------------------------- constants
N_NODES = 100000
F_IN = 256
HID = 16
HEADS = 8
N_CLASSES = 16
NEG_SLOPE = 0.2
NC = 8                      # cores
CH = 128                    # dst nodes per chunk
BLK = 2                     # chunks per block (gather granularity)
GMAX = 8                    # max 128-edge tiles per dma_gather instruction
ROW1 = 192                  # f32 per L1 table row (h 128 | el 8 | pad)
ROW2 = 64                   # f32 per L2 table row (hh 16 | el2 1 | pad)
WIN_EDGE = 65536            # src window split (int16 gather idx)


WIN = 32768  # max rows addressable by one (non-negative) int16 gather window


def _win_bases(n_nodes, win_edge=None):
    """Positive-index windows: window w covers rows [w*WIN, min((w+1)*WIN, n))."""
    nwin = (n_nodes + WIN - 1) // WIN
    bases = [w * WIN for w in range(nwin)]
    return bases, bases


def _wrap16(v):
    # [n] -> [128, n//16] int16; idx i at [i%16, i//16], replicated over groups
    n = v.shape[0]
    assert n % 16 == 0
    a = v.reshape(n // 16, 16).T.astype(np.int16)      # [16, n//16]
    return np.ascontiguousarray(np.tile(a, (8, 1)))    # [128, n//16]


def host_prep(src, dst, n_nodes=N_NODES, nc=NC, ch=CH, win_edge=WIN_EDGE):
    """Partition/sort/pad edges; build gather index + slot arrays per core."""
    sh = n_nodes // nc
    nchunk = (sh + ch - 1) // ch
    bases, wstarts = _win_bases(n_nodes)
    nwin = len(bases)

    per_core = []
    cnt = np.zeros((nc, nchunk, nwin), dtype=np.int64)
    for c in range(nc):
        m = (dst // sh) == c
        es, ed = src[m].astype(np.int64), (dst[m] - c * sh).astype(np.int64)
        chunk = ed // ch
        bank = np.zeros_like(es)
        for w in range(1, nwin):
            bank[es >= wstarts[w]] = w
        order = np.lexsort((bank, chunk))
        es, ed, chunk, bank = es[order], ed[order], chunk[order], bank[order]
        per_core.append((es, ed))
        for k in range(nchunk):
            km = chunk == k
            for w in range(nwin):
                cnt[c, k, w] = np.count_nonzero(km & (bank == w))

    # uniform tile counts: max over cores, ceil to 128-edge tiles
    T = np.maximum(np.ceil(cnt.max(axis=0) / 128.0), 1).astype(np.int64)

    nblk = (nchunk + BLK - 1) // BLK
    # stream layout: for blk: for w: for k in blk: seg(k, w) of T[k,w] tiles
    seg_off = {}
    gath = []             # (block, window, tile_off, ntiles)
    toff = 0
    for b in range(nblk):
        ks = list(range(b * BLK, min((b + 1) * BLK, nchunk)))
        for w in range(nwin):
            g0 = toff
            for k in ks:
                seg_off[(k, w)] = toff
                toff += int(T[k, w])
            gath.append((b, w, g0, toff - g0))
    ntile = toff
    ne_pad = ntile * 128

    gidx = np.zeros((nc, ne_pad), dtype=np.int16)
    didx = np.zeros((nc, ne_pad), dtype=np.int16)
    slot = np.full((nc, ne_pad), -1, dtype=np.int32)
    for c in range(nc):
        es, ed = per_core[c]
        p = 0
        ptr = {}
        for k in range(nchunk):
            for w in range(nwin):
                n = int(cnt[c, k, w])
                ptr[(k, w)] = (p, p + n)
                p += n
        for (k, w), off in seg_off.items():
            a, bnd = ptr[(k, w)]
            n = bnd - a
            pos = off * 128
            gidx[c, pos:pos + n] = (es[a:bnd] - bases[w]).astype(np.int16)
            didx[c, pos:pos + n] = ed[a:bnd].astype(np.int16)
            slot[c, pos:pos + n] = (ed[a:bnd] % ch).astype(np.int32)

    chunk_tiles = []
    for k in range(nchunk):
        tl = []
        for w in range(nwin):
            off = seg_off[(k, w)]
            tl.extend(range(off, off + int(T[k, w])))
        chunk_tiles.append(tl)

    return dict(
        n_nodes=n_nodes, sh=sh, nchunk=nchunk, nwin=nwin, bases=bases,
        nblk=nblk, ntile=ntile, ne_pad=ne_pad, gath=gath,
        chunk_tiles=chunk_tiles, seg_off=seg_off, T=T,
        gidx=gidx, didx=didx, slot=slot,
    )


# ------------------------------------------------------------- bass program
def build_program(meta, f_in, hid, heads, n_classes):
    from contextlib import ExitStack

    import concourse.tile as tile
    from concourse import bacc, mybir

    dt = mybir.dt
    f32, bf16, i16, i32 = dt.float32, dt.bfloat16, dt.int16, dt.int32
    AF = mybir.ActivationFunctionType
    OP = mybir.AluOpType
    AX = mybir.AxisListType

    n_nodes = meta["n_nodes"]
    sh, nchunk, nwin = meta["sh"], meta["nchunk"], meta["nwin"]
    nblk, ntile, ne_pad = meta["nblk"], meta["ntile"], meta["ne_pad"]
    gath, chunk_tiles = meta["gath"], meta["chunk_tiles"]
    wbase = meta["bases"]
    hd = heads * hid
    kt = f_in // 128
    ntile_a = (sh + 127) // 128
    n_last = sh - (ntile_a - 1) * 128

    nc_ = bacc.Bacc("TRN2", target_bir_lowering=False, debug=False,
                    num_devices=NC)

    def din(name, shape, dtype):
        return nc_.dram_tensor(name, list(shape), dtype,
                               kind="ExternalInput").ap()

    featT = din("featT", [f_in, sh], f32)
    W1 = din("W1", [f_in, hd], f32)
    alrep = din("alrep", [128, hd], f32)
    arrep = din("arrep", [128, hd], f32)
    b1rep = din("b1rep", [128, hd], f32)
    W2a = din("W2a", [hd, n_classes + 2], f32)
    b2rep = din("b2rep", [128, n_classes], f32)
    gidx_d = din("gidx", [128, ne_pad // 16], i16)
    didx_d = din("didx", [128, ne_pad // 16], i16)
    slot_d = din("slot", [128, ntile], f32)
    out_d = nc_.dram_tensor("out", [sh, n_classes], f32,
                            kind="ExternalOutput").ap()

    htab = nc_.dram_tensor("htab", [n_nodes, ROW1], f32).ap()
    l2tab = nc_.dram_tensor("l2tab", [n_nodes, ROW2], f32).ap()
    hshard = nc_.dram_tensor("hshard", [sh, ROW1], f32).ap()
    l2shard = nc_.dram_tensor("l2shard", [sh, ROW2], f32).ap()
    ertab = nc_.dram_tensor("ertab", [sh, 64], f32).ap()
    er2tab = nc_.dram_tensor("er2tab", [sh, 64], f32).ap()

    replica = [list(range(NC))]

    with tile.TileContext(nc_) as tc:
        nc = tc.nc
        with ExitStack() as cctx:
            cpool = cctx.enter_context(tc.tile_pool(name="const", bufs=1))
            w1_sb = cpool.tile([128, kt * hd], f32, tag="w1")
            for k in range(kt):
                nc.sync.dma_start(w1_sb[:, k * hd:(k + 1) * hd],
                                  W1[k * 128:(k + 1) * 128, :])
            al_sb = cpool.tile([128, hd], f32, tag="al")
            nc.sync.dma_start(al_sb[:], alrep[:])
            ar_sb = cpool.tile([128, hd], f32, tag="ar")
            nc.sync.dma_start(ar_sb[:], arrep[:])
            b1_sb = cpool.tile([128, hd], f32, tag="b1")
            nc.sync.dma_start(b1_sb[:], b1rep[:])
            w2_sb = cpool.tile([hd, n_classes + 2], f32, tag="w2")
            nc.sync.dma_start(w2_sb[:], W2a[:])
            b2_sb = cpool.tile([128, n_classes], f32, tag="b2")
            nc.sync.dma_start(b2_sb[:], b2rep[:])
            iota_sb = cpool.tile([128, 128], f32, tag="iota")
            nc.gpsimd.iota(iota_sb[:], pattern=[[1, 128]], base=0,
                           channel_multiplier=0,
                           allow_small_or_imprecise_dtypes=True)
            iota_p = cpool.tile([128, 1], f32, tag="iotap")
            nc.gpsimd.iota(iota_p[:], pattern=[[0, 1]], base=0,
                           channel_multiplier=1,
                           allow_small_or_imprecise_dtypes=True)
            ident_sb = cpool.tile([128, 128], f32, tag="ident")
            nc.vector.tensor_scalar(out=ident_sb[:], in0=iota_sb[:],
                                    scalar1=iota_p[:], scalar2=None,
                                    op0=OP.is_equal)

            # ---------------- phase A ----------------
            with ExitStack() as actx:
                apool = actx.enter_context(tc.tile_pool(name="phA", bufs=3))
                apsum = actx.enter_context(
                    tc.tile_pool(name="phAps", bufs=3, space="PSUM"))
                for t in range(ntile_a):
                    m = 128 if t < ntile_a - 1 else n_last
                    ft = apool.tile([128, kt, 128], f32, tag="ft")
                    for k in range(kt):
                        nc.sync.dma_start(
                            ft[:, k, :m],
                            featT[k * 128:(k + 1) * 128,
                                  t * 128:t * 128 + m])
                    ps = apsum.tile([128, hd], f32, tag="hps")
                    for k in range(kt):
                        nc.tensor.matmul(
                            ps[:m, :], lhsT=ft[:, k, :m],
                            rhs=w1_sb[:, k * hd:(k + 1) * hd],
                            start=(k == 0), stop=(k == kt - 1))
                    row = apool.tile([128, ROW1], f32, tag="row")
                    nc.scalar.copy(row[:m, 0:hd], ps[:m, :])
                    tmp = apool.tile([128, hd], f32, tag="tmpa")
                    nc.vector.tensor_mul(tmp[:m, :], ps[:m, :], al_sb[:m, :])
                    nc.vector.tensor_reduce(
                        out=row[:m, hd:hd + heads],
                        in_=tmp[:m, :].rearrange("p (h d) -> p h d", h=heads),
                        axis=AX.X, op=OP.add)
                    ert = apool.tile([128, 64], f32, tag="ert")
                    nc.vector.tensor_mul(tmp[:m, :], ps[:m, :], ar_sb[:m, :])
                    nc.vector.tensor_reduce(
                        out=ert[:m, 0:heads],
                        in_=tmp[:m, :].rearrange("p (h d) -> p h d", h=heads),
                        axis=AX.X, op=OP.add)
                    nc.vector.memset(row[:m, hd + heads:ROW1], 0.0)
                    nc.vector.memset(ert[:m, heads:64], 0.0)
                    nc.sync.dma_start(hshard[t * 128:t * 128 + m, :],
                                      row[:m, :])
                    nc.sync.dma_start(ertab[t * 128:t * 128 + m, :],
                                      ert[:m, :])

            nc.gpsimd.collective_compute(
                "AllGather", OP.bypass, replica_groups=replica,
                ins=[hshard.opt()], outs=[htab.opt()])

            # ---------------- edge phases ----------------
            def post_chunk_l1(k, ps, mrows, ppost, pps2):
                fw, sw = hd, heads
                den = ppost.tile([128, sw], f32, tag="den")
                nc.vector.tensor_scalar_max(den[:mrows, :],
                                            ps[:mrows, fw:fw + sw], 1e-30)
                rec = ppost.tile([128, sw], f32, tag="rec")
                nc.vector.reciprocal(rec[:mrows, :], den[:mrows, :])
                h2 = ppost.tile([128, fw], f32, tag="h2")
                nc.vector.tensor_mul(
                    h2[:mrows, :].rearrange("p (s d) -> p s d", s=sw),
                    ps[:mrows, 0:fw].rearrange("p (s d) -> p s d", s=sw),
                    rec[:mrows, :].unsqueeze(2)
                    .broadcast_to([mrows, sw, fw // sw]))
                nc.vector.tensor_add(h2[:mrows, :], h2[:mrows, :],
                                     b1_sb[:mrows, :])
                mn = ppost.tile([128, fw], f32, tag="mn")
                nc.vector.tensor_scalar_min(mn[:mrows, :], h2[:mrows, :], 0.0)
                nc.scalar.activation(mn[:mrows, :], mn[:mrows, :], AF.Exp)
                nc.vector.scalar_tensor_tensor(
                    out=h2[:mrows, :], in0=h2[:mrows, :], scalar=0.0,
                    in1=mn[:mrows, :], op0=OP.max, op1=OP.add)
                nc.vector.tensor_scalar_sub(h2[:mrows, :], h2[:mrows, :], 1.0)
                # L2 node phase
                pst = pps2.tile([128, 128], f32, tag="pst")
                nc.tensor.transpose(pst[:, :mrows], h2[:mrows, :],
                                    ident_sb[:mrows, :mrows])
                h2T = ppost.tile([128, 128], f32, tag="h2T")
                nc.scalar.copy(h2T[:, :mrows], pst[:, :mrows])
                ps2 = pps2.tile([128, n_classes + 2], f32, tag="hh")
                nc.tensor.matmul(ps2[:mrows, :], lhsT=h2T[:, :mrows],
                                 rhs=w2_sb[:], start=True, stop=True)
                l2r = ppost.tile([128, ROW2], f32, tag="l2r")
                nc.scalar.copy(l2r[:mrows, 0:n_classes + 1],
                               ps2[:mrows, 0:n_classes + 1])
                nc.vector.memset(l2r[:mrows, n_classes + 1:ROW2], 0.0)
                er2r = ppost.tile([128, 64], f32, tag="er2r")
                nc.scalar.copy(er2r[:mrows, 0:1],
                               ps2[:mrows, n_classes + 1:n_classes + 2])
                nc.vector.memset(er2r[:mrows, 1:64], 0.0)
                nc.sync.dma_start(l2shard[k * CH:k * CH + mrows, :],
                                  l2r[:mrows, :])
                nc.sync.dma_start(er2tab[k * CH:k * CH + mrows, :],
                                  er2r[:mrows, :])

            def post_chunk_l2(k, ps, mrows, ppost):
                fw = n_classes
                den = ppost.tile([128, 1], f32, tag="den2")
                nc.vector.tensor_scalar_max(den[:mrows, :],
                                            ps[:mrows, fw:fw + 1], 1e-30)
                rec = ppost.tile([128, 1], f32, tag="rec2")
                nc.vector.reciprocal(rec[:mrows, :], den[:mrows, :])
                xx = ppost.tile([128, fw], f32, tag="xx")
                nc.vector.tensor_scalar(out=xx[:mrows, :],
                                        in0=ps[:mrows, 0:fw],
                                        scalar1=rec[:mrows, :], scalar2=None,
                                        op0=OP.mult)
                nc.vector.tensor_add(xx[:mrows, :], xx[:mrows, :],
                                     b2_sb[:mrows, :])
                rmax = ppost.tile([128, 1], f32, tag="rmax")
                nc.vector.tensor_reduce(out=rmax[:mrows, :], in_=xx[:mrows, :],
                                        axis=AX.X, op=OP.max)
                nc.vector.tensor_scalar(out=xx[:mrows, :], in0=xx[:mrows, :],
                                        scalar1=rmax[:mrows, :], scalar2=None,
                                        op0=OP.subtract)
                exs = ppost.tile([128, fw], f32, tag="exs")
                ssum = ppost.tile([128, 1], f32, tag="ssum")
                nc.scalar.activation(exs[:mrows, :], xx[:mrows, :], AF.Exp,
                                     accum_out=ssum[:mrows, :])
                lss = ppost.tile([128, 1], f32, tag="lss")
                nc.scalar.activation(lss[:mrows, :], ssum[:mrows, :], AF.Ln)
                nc.vector.tensor_scalar(out=xx[:mrows, :], in0=xx[:mrows, :],
                                        scalar1=lss[:mrows, :], scalar2=None,
                                        op0=OP.subtract)
                nc.sync.dma_start(out_d[k * CH:k * CH + mrows, :],
                                  xx[:mrows, :])

            def edge_phase(layer):
                if layer == 1:
                    tab, ertb, rw, fw, sw = htab, ertab, ROW1, hd, heads
                else:
                    tab, ertb, rw, fw, sw = l2tab, er2tab, ROW2, n_classes, 1
                nw = fw + sw

                gblocks = {}
                for (b, w, g0, nt) in gath:
                    gblocks.setdefault(b, []).append((w, g0, nt))

                with ExitStack() as ectx:
                    pool = ectx.enter_context(
                        tc.tile_pool(name=f"edge{layer}", bufs=2))
                    pps = ectx.enter_context(
                        tc.tile_pool(name=f"eps{layer}", bufs=3,
                                     space="PSUM"))
                    ppost = ectx.enter_context(
                        tc.tile_pool(name=f"post{layer}", bufs=2))
                    pps2 = ectx.enter_context(
                        tc.tile_pool(name=f"ep2{layer}", bufs=2,
                                     space="PSUM"))
                    for b in range(nblk):
                        segs = gblocks[b]
                        t0 = segs[0][1]
                        tb = sum(s[2] for s in segs)
                        gt = pool.tile([128, tb, rw], f32, tag="gt")
                        for (w, g0, nt) in segs:
                            if nt == 0:
                                continue
                            ii = pool.tile([128, nt * 8], i16, tag="gi")
                            nc.sync.dma_start(
                                ii[:], gidx_d[:, g0 * 8:(g0 + nt) * 8])
                            wend = min(wbase[w] + 32768, n_nodes)
                            for s0 in range(0, nt, GMAX):
                                sn = min(GMAX, nt - s0)
                                nc.gpsimd.dma_gather(
                                    out_ap=gt[:, g0 - t0 + s0:
                                              g0 - t0 + s0 + sn, :],
                                    in_ap=tab[wbase[w]:wend, :],
                                    idxs_ap=ii[:, s0 * 8:(s0 + sn) * 8],
                                    num_idxs=sn * 128,
                                    num_idxs_reg=sn * 128, elem_size=rw,
                                    queue_num=0)
                        di = pool.tile([128, tb * 8], i16, tag="di")
                        nc.sync.dma_start(di[:],
                                          didx_d[:, t0 * 8:(t0 + tb) * 8])
                        erg = pool.tile([128, tb, 64], f32, tag="erg")
                        for s0 in range(0, tb, GMAX):
                            sn = min(GMAX, tb - s0)
                            nc.gpsimd.dma_gather(
                                out_ap=erg[:, s0:s0 + sn, :], in_ap=ertb[:],
                                idxs_ap=di[:, s0 * 8:(s0 + sn) * 8],
                                num_idxs=sn * 128, num_idxs_reg=sn * 128,
                                elem_size=64, queue_num=0)
                        sl = pool.tile([128, tb], f32, tag="sl")
                        nc.sync.dma_start(sl[:], slot_d[:, t0:t0 + tb])
                        B = pool.tile([128, tb, 128], bf16, tag="B")
                        nc.vector.tensor_tensor(
                            out=B[:],
                            in0=iota_sb[:].unsqueeze(1)
                            .broadcast_to([128, tb, 128]),
                            in1=sl[:].unsqueeze(2)
                            .broadcast_to([128, tb, 128]),
                            op=OP.is_equal)
                        ex = pool.tile([128, tb, sw], f32, tag="ex")
                        nc.vector.tensor_add(ex[:], gt[:, :, fw:fw + sw],
                                             erg[:, :, 0:sw])
                        nc.vector.scalar_tensor_tensor(
                            out=ex[:], in0=ex[:], scalar=NEG_SLOPE,
                            in1=ex[:], op0=OP.mult, op1=OP.max)
                        nc.scalar.activation(ex[:], ex[:], AF.Exp)
                        comb = pool.tile([128, tb, nw], bf16, tag="comb")
                        nc.scalar.copy(comb[:, :, fw:fw + sw], ex[:])
                        nc.vector.tensor_mul(
                            comb[:, :, 0:fw].rearrange(
                                "p t (s d) -> p t s d", s=sw),
                            gt[:, :, 0:fw].rearrange(
                                "p t (s d) -> p t s d", s=sw),
                            ex[:].unsqueeze(3)
                            .broadcast_to([128, tb, sw, fw // sw]))
                        for k in range(b * BLK, min((b + 1) * BLK, nchunk)):
                            tl = chunk_tiles[k]
                            ps = pps.tile([128, nw], f32, tag="agg")
                            for j, t in enumerate(tl):
                                nc.tensor.matmul(
                                    ps[:], lhsT=B[:, t - t0, :],
                                    rhs=comb[:, t - t0, :],
                                    start=(j == 0), stop=(j == len(tl) - 1))
                            mrows = min(CH, sh - k * CH)
                            if layer == 1:
                                post_chunk_l1(k, ps, mrows, ppost, pps2)
                            else:
                                post_chunk_l2(k, ps, mrows, ppost)

            edge_phase(1)
            nc.gpsimd.collective_compute(
                "AllGather", OP.bypass, replica_groups=replica,
                ins=[l2shard.opt()], outs=[l2tab.opt()])
            edge_phase(2)

    nc_.compile()
    return nc_


# ------------------------------------------------------------------ driver
def make_in_maps(meta, feat, W1, al1, ar1, b1, W2, al2, ar2, b2):
    sh = meta["sh"]
    alrep = np.tile(al1.reshape(1, -1), (128, 1)).astype(np.float32)
    arrep = np.tile(ar1.reshape(1, -1), (128, 1)).astype(np.float32)
    b1rep = np.tile(b1.reshape(1, -1), (128, 1)).astype(np.float32)
    W2a = np.concatenate(
        [W2, W2 @ al2.reshape(-1, 1), W2 @ ar2.reshape(-1, 1)],
        axis=1).astype(np.float32)
    b2rep = np.tile(b2.reshape(1, -1), (128, 1)).astype(np.float32)
    in_maps = []
    for c in range(NC):
        in_maps.append({
            "featT": np.ascontiguousarray(feat[c * sh:(c + 1) * sh, :].T),
            "W1": W1, "alrep": alrep, "arrep": arrep, "b1rep": b1rep,
            "W2a": W2a, "b2rep": b2rep,
            "gidx": _wrap16(meta["gidx"][c]),
            "didx": _wrap16(meta["didx"][c]),
            "slot": np.ascontiguousarray(
                meta["slot"][c].reshape(-1, 128).T).astype(np.float32),
        })
    return in_maps


class Runner:
    """Builds the SPMD program once; exposes a repeatable timed executor."""

    def __init__(self, meta, f_in):
        self.meta = meta
        self.nc = build_program(meta, f_in, HID, HEADS, N_CLASSES)
        self._fn = None
        self.repeat = 1

    repeat = 1

    def _lower(self):
        import jax
        import numpy as _np
        from jax.sharding import Mesh, PartitionSpec
        from jax.experimental.shard_map import shard_map
        from concourse import mybir
        from concourse.bass2jax import _bass_exec_p, install_neuronx_cc_hook

        install_neuronx_cc_hook()
        nc = self.nc
        in_names, out_names, out_avals, zero_outs = [], [], [], []
        partition_name = (nc.partition_id_tensor.name
                          if nc.partition_id_tensor else None)
        for alloc in nc.m.functions[0].allocations:
            if not isinstance(alloc, mybir.MemoryLocationSet):
                continue
            name = alloc.memorylocations[0].name
            if alloc.kind == "ExternalInput":
                if name != partition_name:
                    in_names.append(name)
            elif alloc.kind == "ExternalOutput":
                shape = tuple(alloc.tensor_shape)
                dtype = mybir.dt.np(alloc.dtype)
                out_names.append(name)
                out_avals.append(jax.core.ShapedArray(shape, dtype))
                zero_outs.append(_np.zeros(shape, dtype))
        n_params = len(in_names)
        n_outs = len(out_avals)
        all_in_names = list(in_names) + list(out_names)
        if partition_name is not None:
            all_in_names.append(partition_name)

        repeat = self.repeat

        def _body(*args):
            ins = list(args[:n_params])
            zouts = list(args[n_params:])
            outs = None
            for _ in range(repeat):
                operands = ins + zouts
                if partition_name is not None:
                    from concourse.bass2jax import partition_id_tensor
                    operands.append(partition_id_tensor())
                outs = _bass_exec_p.bind(
                    *operands, out_avals=tuple(out_avals),
                    in_names=tuple(all_in_names), out_names=tuple(out_names),
                    lowering_input_output_aliases=(),
                    sim_require_finite=True, sim_require_nnan=True, nc=nc)
                zouts = list(outs)   # chain: serialize + reuse as out bufs
            return tuple(outs)

        devices = jax.devices()[:NC]
        mesh = Mesh(_np.asarray(devices), ("core",))
        in_specs = (PartitionSpec("core"),) * (n_params + n_outs)
        out_specs = (PartitionSpec("core"),) * n_outs
        self._fn = jax.jit(
            shard_map(_body, mesh=mesh, in_specs=in_specs,
                      out_specs=out_specs, check_rep=False),
            keep_unused=True)
        self._in_names = in_names
        self._out_names = out_names
        self._out_avals = out_avals
        self._zero_outs = zero_outs
        self._mesh = mesh
        self._in_specs = in_specs

    def prepare(self, in_maps):
        import jax
        import numpy as _np
        from jax.sharding import NamedSharding, PartitionSpec
        if self._fn is None:
            self._lower()
        concat_in = [
            _np.concatenate([in_maps[c][name] for c in range(NC)], axis=0)
            for name in self._in_names]
        concat_zeros = [
            _np.zeros((NC * z.shape[0], *z.shape[1:]), z.dtype)
            for z in self._zero_outs]
        shd = NamedSharding(self._mesh, PartitionSpec("core"))
        self._args = [jax.device_put(a, shd) for a in concat_in + concat_zeros]
        jax.block_until_ready(self._args)

    def run(self):
        import jax
        out = self._fn(*self._args)
        out = jax.block_until_ready(out)
        import numpy as _np
        res = _np.asarray(out[self._out_names.index("out")])
        sh = self._out_avals[self._out_names.index("out")].shape
        return res.reshape(NC, *sh).reshape(NC * sh[0], *sh[1:])


_RUNNER = None


def get_runner(feat, src, dst):
    global _RUNNER
    n, f_in = feat.shape
    meta = host_prep(np.asarray(src, np.int32), np.asarray(dst, np.int32),
                     n_nodes=n)
    _RUNNER = Runner(meta, f_in)
    return _RUNNER


def kernel(feat, src, dst, W1, al1, ar1, b1, W2, al2, ar2, b2):
    feat = np.asarray(feat, dtype=np.float32)
    src = np.asarray(src, dtype=np.int32)
    dst = np.asarray(dst, dtype=np.int32)
    args = [np.asarray(x, np.float32)
            for x in (W1, al1, ar1, b1, W2, al2, ar2, b2)]
    r = _RUNNER if _RUNNER is not None else get_runner(feat, src, dst)
    in_maps = make_in_maps(r.meta, feat, *args)
    r.prepare(in_maps)
    return r.run()


kernel.last_exec_time_ns = None


# revision 17
# speedup vs baseline: 35.2569x; 1.0927x over previous
"""Bass/Trainium2 kernel for a 2-layer GAT (GATConv x2 + log_softmax) on 8 NeuronCores.

Strategy (edge/data parallel, dst-sharded):
  - Nodes sharded 8 ways by id; core c owns dst nodes [c*SH, (c+1)*SH).
  - Phase A (per core): h = feat_shard @ W1, el/er per node.  [h|el] written to
    a gather table (768B rows), AllGathered to every core.  er kept in a local
    256B-row table.
  - Phase B (L1 edges, partitioned by dst owner): per 128-edge tile,
    dma_gather h[src] rows from HBM, dma_gather er[dst] rows, build a one-hot
    B[edge, dst_slot] matrix on DVE, and aggregate messages with PE matmuls:
    psum[slot, :] += B^T @ [exp(leaky(el+er)) * h[src] | exp(...)].
    Normalize by the attention-sum column, add bias, ELU -> h2.
  - L2 node phase fused per chunk: hh|el2|er2 = h2T @ [W2|W2@al2|W2@ar2]
    (single matmul after a PE transpose), written to the L2 table + AllGather.
  - Phase C: same edge pipeline on 256B rows, then log_softmax, output shard.

Edges per core are sorted by (dst chunk of 128 slots, src window), padded to
128-edge tiles with counts uniform across cores so one SPMD program serves all
8 cores.  src gather indices are int16 with 2 overlapping windows (the base
points into the middle of the table; negative indices address below it).
No segment-max is needed: attention logits are O(1) so exp() cannot overflow.
"""

import os
import sys

import numpy as np

sys.path.insert(0, "/opt/trn_rl_repo")

# ---------------------------------------------------------------- constants
N_NODES = 100000
F_IN = 256
HID = 16
HEADS = 8
N_CLASSES = 16
NEG_SLOPE = 0.2
NC = 8                      # cores
CH = 128                    # dst nodes per chunk
BLK = 2                     # chunks per block (gather granularity)
GMAX = 8                    # max 128-edge tiles per dma_gather instruction
ROW1 = 192                  # f32 per L1 table row (h 128 | el 8 | pad)
ROW2 = 64                   # f32 per L2 table row (hh 16 | el2 1 | pad)
WIN_EDGE = 65536            # src window split (int16 gather idx)


WIN = 32768  # max rows addressable by one (non-negative) int16 gather window


def _win_bases(n_nodes, win_edge=None):
    """Positive-index windows: window w covers rows [w*WIN, min((w+1)*WIN, n))."""
    nwin = (n_nodes + WIN - 1) // WIN
    bases = [w * WIN for w in range(nwin)]
    return bases, bases


def _wrap16(v):
    # [n] -> [128, n//16] int16; idx i at [i%16, i//16], replicated over groups
    n = v.shape[0]
    assert n % 16 == 0
    a = v.reshape(n // 16, 16).T.astype(np.int16)      # [16, n//16]
    return np.ascontiguousarray(np.tile(a, (8, 1)))    # [128, n//16]


def host_prep(src, dst, n_nodes=N_NODES, nc=NC, ch=CH, win_edge=WIN_EDGE):
    """Partition/sort/pad edges; build gather index + slot arrays per core."""
    sh = n_nodes // nc
    nchunk = (sh + ch - 1) // ch
    bases, wstarts = _win_bases(n_nodes)
    nwin = len(bases)

    per_core = []
    cnt = np.zeros((nc, nchunk, nwin), dtype=np.int64)
    for c in range(nc):
        m = (dst // sh) == c
        es, ed = src[m].astype(np.int64), (dst[m] - c * sh).astype(np.int64)
        chunk = ed // ch
        bank = np.zeros_like(es)
        for w in range(1, nwin):
            bank[es >= wstarts[w]] = w
        order = np.lexsort((bank, chunk))
        es, ed, chunk, bank = es[order], ed[order], chunk[order], bank[order]
        per_core.append((es, ed))
        for k in range(nchunk):
            km = chunk == k
            for w in range(nwin):
                cnt[c, k, w] = np.count_nonzero(km & (bank == w))

    # uniform tile counts: max over cores, ceil to 128-edge tiles
    T = np.maximum(np.ceil(cnt.max(axis=0) / 128.0), 1).astype(np.int64)

    nblk = (nchunk + BLK - 1) // BLK
    # stream layout: for blk: for w: for k in blk: seg(k, w) of T[k,w] tiles
    seg_off = {}
    gath = []             # (block, window, tile_off, ntiles)
    toff = 0
    for b in range(nblk):
        ks = list(range(b * BLK, min((b + 1) * BLK, nchunk)))
        for w in range(nwin):
            g0 = toff
            for k in ks:
                seg_off[(k, w)] = toff
                toff += int(T[k, w])
            gath.append((b, w, g0, toff - g0))
    ntile = toff
    ne_pad = ntile * 128

    gidx = np.zeros((nc, ne_pad), dtype=np.int16)
    didx = np.zeros((nc, ne_pad), dtype=np.int16)
    slot = np.full((nc, ne_pad), -1, dtype=np.int32)
    for c in range(nc):
        es, ed = per_core[c]
        p = 0
        ptr = {}
        for k in range(nchunk):
            for w in range(nwin):
                n = int(cnt[c, k, w])
                ptr[(k, w)] = (p, p + n)
                p += n
        for (k, w), off in seg_off.items():
            a, bnd = ptr[(k, w)]
            n = bnd - a
            pos = off * 128
            gidx[c, pos:pos + n] = (es[a:bnd] - bases[w]).astype(np.int16)
            didx[c, pos:pos + n] = ed[a:bnd].astype(np.int16)
            slot[c, pos:pos + n] = (ed[a:bnd] % ch).astype(np.int32)

    chunk_tiles = []
    for k in range(nchunk):
        tl = []
        for w in range(nwin):
            off = seg_off[(k, w)]
            tl.extend(range(off, off + int(T[k, w])))
        chunk_tiles.append(tl)

    return dict(
        n_nodes=n_nodes, sh=sh, nchunk=nchunk, nwin=nwin, bases=bases,
        nblk=nblk, ntile=ntile, ne_pad=ne_pad, gath=gath,
        chunk_tiles=chunk_tiles, seg_off=seg_off, T=T,
        gidx=gidx, didx=didx, slot=slot,
    )


# ------------------------------------------------------------- bass program
def build_program(meta, f_in, hid, heads, n_classes):
    from contextlib import ExitStack

    import concourse.tile as tile
    from concourse import bacc, mybir

    dt = mybir.dt
    f32, bf16, i16, i32 = dt.float32, dt.bfloat16, dt.int16, dt.int32
    AF = mybir.ActivationFunctionType
    OP = mybir.AluOpType
    AX = mybir.AxisListType

    n_nodes = meta["n_nodes"]
    sh, nchunk, nwin = meta["sh"], meta["nchunk"], meta["nwin"]
    nblk, ntile, ne_pad = meta["nblk"], meta["ntile"], meta["ne_pad"]
    gath, chunk_tiles = meta["gath"], meta["chunk_tiles"]
    wbase = meta["bases"]
    hd = heads * hid
    kt = f_in // 128
    ntile_a = (sh + 127) // 128
    n_last = sh - (ntile_a - 1) * 128

    nc_ = bacc.Bacc("TRN2", target_bir_lowering=False, debug=False,
                    num_devices=NC)

    def din(name, shape, dtype):
        return nc_.dram_tensor(name, list(shape), dtype,
                               kind="ExternalInput").ap()

    featT = din("featT", [f_in, sh], f32)
    W1 = din("W1", [f_in, hd], f32)
    alrep = din("alrep", [128, hd], f32)
    arrep = din("arrep", [128, hd], f32)
    b1rep = din("b1rep", [128, hd], f32)
    W2a = din("W2a", [hd, n_classes + 2], f32)
    b2rep = din("b2rep", [128, n_classes], f32)
    gidx_d = din("gidx", [128, ne_pad // 16], i16)
    didx_d = din("didx", [128, ne_pad // 16], i16)
    slot_d = din("slot", [128, ntile], f32)
    out_d = nc_.dram_tensor("out", [sh, n_classes], f32,
                            kind="ExternalOutput").ap()

    htab = nc_.dram_tensor("htab", [n_nodes, ROW1], f32).ap()
    l2tab = nc_.dram_tensor("l2tab", [n_nodes, ROW2], f32).ap()
    hshard = nc_.dram_tensor("hshard", [sh, ROW1], f32).ap()
    l2shard = nc_.dram_tensor("l2shard", [sh, ROW2], f32).ap()
    ertab = nc_.dram_tensor("ertab", [sh, 64], f32).ap()
    er2tab = nc_.dram_tensor("er2tab", [sh, 64], f32).ap()

    replica = [list(range(NC))]

    with tile.TileContext(nc_) as tc:
        nc = tc.nc
        with ExitStack() as cctx:
            cpool = cctx.enter_context(tc.tile_pool(name="const", bufs=1))
            w1_sb = cpool.tile([128, kt * hd], f32, tag="w1")
            for k in range(kt):
                nc.sync.dma_start(w1_sb[:, k * hd:(k + 1) * hd],
                                  W1[k * 128:(k + 1) * 128, :])
            al_sb = cpool.tile([128, hd], f32, tag="al")
            nc.sync.dma_start(al_sb[:], alrep[:])
            ar_sb = cpool.tile([128, hd], f32, tag="ar")
            nc.sync.dma_start(ar_sb[:], arrep[:])
            b1_sb = cpool.tile([128, hd], f32, tag="b1")
            nc.sync.dma_start(b1_sb[:], b1rep[:])
            w2_sb = cpool.tile([hd, n_classes + 2], f32, tag="w2")
            nc.sync.dma_start(w2_sb[:], W2a[:])
            b2_sb = cpool.tile([128, n_classes], f32, tag="b2")
            nc.sync.dma_start(b2_sb[:], b2rep[:])
            iota_sb = cpool.tile([128, 128], f32, tag="iota")
            nc.gpsimd.iota(iota_sb[:], pattern=[[1, 128]], base=0,
                           channel_multiplier=0,
                           allow_small_or_imprecise_dtypes=True)
            iota_p = cpool.tile([128, 1], f32, tag="iotap")
            nc.gpsimd.iota(iota_p[:], pattern=[[0, 1]], base=0,
                           channel_multiplier=1,
                           allow_small_or_imprecise_dtypes=True)
            ident_sb = cpool.tile([128, 128], f32, tag="ident")
            nc.vector.tensor_scalar(out=ident_sb[:], in0=iota_sb[:],
                                    scalar1=iota_p[:], scalar2=None,
                                    op0=OP.is_equal)

            # ---------------- phase A ----------------
            with ExitStack() as actx:
                apool = actx.enter_context(tc.tile_pool(name="phA", bufs=3))
                apsum = actx.enter_context(
                    tc.tile_pool(name="phAps", bufs=3, space="PSUM"))
                for t in range(ntile_a):
                    m = 128 if t < ntile_a - 1 else n_last
                    ft = apool.tile([128, kt, 128], f32, tag="ft")
                    for k in range(kt):
                        nc.sync.dma_start(
                            ft[:, k, :m],
                            featT[k * 128:(k + 1) * 128,
                                  t * 128:t * 128 + m])
                    ps = apsum.tile([128, hd], f32, tag="hps")
                    for k in range(kt):
                        nc.tensor.matmul(
                            ps[:m, :], lhsT=ft[:, k, :m],
                            rhs=w1_sb[:, k * hd:(k + 1) * hd],
                            start=(k == 0), stop=(k == kt - 1))
                    row = apool.tile([128, ROW1], f32, tag="row")
                    nc.scalar.copy(row[:m, 0:hd], ps[:m, :])
                    tmp = apool.tile([128, hd], f32, tag="tmpa")
                    nc.vector.tensor_mul(tmp[:m, :], ps[:m, :], al_sb[:m, :])
                    nc.vector.tensor_reduce(
                        out=row[:m, hd:hd + heads],
                        in_=tmp[:m, :].rearrange("p (h d) -> p h d", h=heads),
                        axis=AX.X, op=OP.add)
                    ert = apool.tile([128, 64], f32, tag="ert")
                    nc.vector.tensor_mul(tmp[:m, :], ps[:m, :], ar_sb[:m, :])
                    nc.vector.tensor_reduce(
                        out=ert[:m, 0:heads],
                        in_=tmp[:m, :].rearrange("p (h d) -> p h d", h=heads),
                        axis=AX.X, op=OP.add)
                    nc.vector.memset(row[:m, hd + heads:ROW1], 0.0)
                    nc.vector.memset(ert[:m, heads:64], 0.0)
                    nc.sync.dma_start(hshard[t * 128:t * 128 + m, :],
                                      row[:m, :])
                    nc.sync.dma_start(ertab[t * 128:t * 128 + m, :],
                                      ert[:m, :])

            nc.gpsimd.collective_compute(
                "AllGather", OP.bypass, replica_groups=replica,
                ins=[hshard.opt()], outs=[htab.opt()])

            # ---------------- edge phases ----------------
            def post_chunk_l1(k, ps, mrows, ppost, pps2):
                fw, sw = hd, heads
                den = ppost.tile([128, sw], f32, tag="den")
                nc.vector.tensor_scalar_max(den[:mrows, :],
                                            ps[:mrows, fw:fw + sw], 1e-30)
                rec = ppost.tile([128, sw], f32, tag="rec")
                nc.vector.reciprocal(rec[:mrows, :], den[:mrows, :])
                h2 = ppost.tile([128, fw], f32, tag="h2")
                nc.vector.tensor_mul(
                    h2[:mrows, :].rearrange("p (s d) -> p s d", s=sw),
                    ps[:mrows, 0:fw].rearrange("p (s d) -> p s d", s=sw),
                    rec[:mrows, :].unsqueeze(2)
                    .broadcast_to([mrows, sw, fw // sw]))
                nc.vector.tensor_add(h2[:mrows, :], h2[:mrows, :],
                                     b1_sb[:mrows, :])
                mn = ppost.tile([128, fw], f32, tag="mn")
                nc.vector.tensor_scalar_min(mn[:mrows, :], h2[:mrows, :], 0.0)
                nc.scalar.activation(mn[:mrows, :], mn[:mrows, :], AF.Exp)
                nc.vector.scalar_tensor_tensor(
                    out=h2[:mrows, :], in0=h2[:mrows, :], scalar=0.0,
                    in1=mn[:mrows, :], op0=OP.max, op1=OP.add)
                nc.vector.tensor_scalar_sub(h2[:mrows, :], h2[:mrows, :], 1.0)
                # L2 node phase
                pst = pps2.tile([128, 128], f32, tag="pst")
                nc.tensor.transpose(pst[:, :mrows], h2[:mrows, :],
                                    ident_sb[:mrows, :mrows])
                h2T = ppost.tile([128, 128], f32, tag="h2T")
                nc.scalar.copy(h2T[:, :mrows], pst[:, :mrows])
                ps2 = pps2.tile([128, n_classes + 2], f32, tag="hh")
                nc.tensor.matmul(ps2[:mrows, :], lhsT=h2T[:, :mrows],
                                 rhs=w2_sb[:], start=True, stop=True)
                l2r = ppost.tile([128, ROW2], f32, tag="l2r")
                nc.scalar.copy(l2r[:mrows, 0:n_classes + 1],
                               ps2[:mrows, 0:n_classes + 1])
                nc.vector.memset(l2r[:mrows, n_classes + 1:ROW2], 0.0)
                er2r = ppost.tile([128, 64], f32, tag="er2r")
                nc.scalar.copy(er2r[:mrows, 0:1],
                               ps2[:mrows, n_classes + 1:n_classes + 2])
                nc.vector.memset(er2r[:mrows, 1:64], 0.0)
                nc.sync.dma_start(l2shard[k * CH:k * CH + mrows, :],
                                  l2r[:mrows, :])
                nc.sync.dma_start(er2tab[k * CH:k * CH + mrows, :],
                                  er2r[:mrows, :])

            def post_chunk_l2(k, ps, mrows, ppost):
                fw = n_classes
                den = ppost.tile([128, 1], f32, tag="den2")
                nc.vector.tensor_scalar_max(den[:mrows, :],
                                            ps[:mrows, fw:fw + 1], 1e-30)
                rec = ppost.tile([128, 1], f32, tag="rec2")
                nc.vector.reciprocal(rec[:mrows, :], den[:mrows, :])
                xx = ppost.tile([128, fw], f32, tag="xx")
                nc.vector.tensor_scalar(out=xx[:mrows, :],
                                        in0=ps[:mrows, 0:fw],
                                        scalar1=rec[:mrows, :], scalar2=None,
                                        op0=OP.mult)
                nc.vector.tensor_add(xx[:mrows, :], xx[:mrows, :],
                                     b2_sb[:mrows, :])
                rmax = ppost.tile([128, 1], f32, tag="rmax")
                nc.vector.tensor_reduce(out=rmax[:mrows, :], in_=xx[:mrows, :],
                                        axis=AX.X, op=OP.max)
                nc.vector.tensor_scalar(out=xx[:mrows, :], in0=xx[:mrows, :],
                                        scalar1=rmax[:mrows, :], scalar2=None,
                                        op0=OP.subtract)
                exs = ppost.tile([128, fw], f32, tag="exs")
                ssum = ppost.tile([128, 1], f32, tag="ssum")
                nc.scalar.activation(exs[:mrows, :], xx[:mrows, :], AF.Exp,
                                     accum_out=ssum[:mrows, :])
                lss = ppost.tile([128, 1], f32, tag="lss")
                nc.scalar.activation(lss[:mrows, :], ssum[:mrows, :], AF.Ln)
                nc.vector.tensor_scalar(out=xx[:mrows, :], in0=xx[:mrows, :],
                                        scalar1=lss[:mrows, :], scalar2=None,
                                        op0=OP.subtract)
                nc.sync.dma_start(out_d[k * CH:k * CH + mrows, :],
                                  xx[:mrows, :])

            def edge_phase(layer):
                if layer == 1:
                    tab, ertb, rw, fw, sw = htab, ertab, ROW1, hd, heads
                else:
                    tab, ertb, rw, fw, sw = l2tab, er2tab, ROW2, n_classes, 1
                nw = fw + sw

                gblocks = {}
                for (b, w, g0, nt) in gath:
                    gblocks.setdefault(b, []).append((w, g0, nt))

                with ExitStack() as ectx:
                    pool = ectx.enter_context(
                        tc.tile_pool(name=f"edge{layer}", bufs=2))
                    pps = ectx.enter_context(
                        tc.tile_pool(name=f"eps{layer}", bufs=3,
                                     space="PSUM"))
                    ppost = ectx.enter_context(
                        tc.tile_pool(name=f"post{layer}", bufs=2))
                    pps2 = ectx.enter_context(
                        tc.tile_pool(name=f"ep2{layer}", bufs=2,
                                     space="PSUM"))
                    for b in range(nblk):
                        segs = gblocks[b]
                        t0 = segs[0][1]
                        tb = sum(s[2] for s in segs)
                        gt = pool.tile([128, tb, rw], f32, tag="gt")
                        for (w, g0, nt) in segs:
                            if nt == 0:
                                continue
                            ii = pool.tile([128, nt * 8], i16, tag="gi")
                            nc.sync.dma_start(
                                ii[:], gidx_d[:, g0 * 8:(g0 + nt) * 8])
                            wend = min(wbase[w] + 32768, n_nodes)
                            for s0 in range(0, nt, GMAX):
                                sn = min(GMAX, nt - s0)
                                nc.gpsimd.dma_gather(
                                    out_ap=gt[:, g0 - t0 + s0:
                                              g0 - t0 + s0 + sn, :],
                                    in_ap=tab[wbase[w]:wend, :],
                                    idxs_ap=ii[:, s0 * 8:(s0 + sn) * 8],
                                    num_idxs=sn * 128,
                                    num_idxs_reg=sn * 128, elem_size=rw,
                                    queue_num=0)
                        di = pool.tile([128, tb * 8], i16, tag="di")
                        nc.sync.dma_start(di[:],
                                          didx_d[:, t0 * 8:(t0 + tb) * 8])
                        erg = pool.tile([128, tb, 64], f32, tag="erg")
                        for s0 in range(0, tb, GMAX):
                            sn = min(GMAX, tb - s0)
                            nc.gpsimd.dma_gather(
                                out_ap=erg[:, s0:s0 + sn, :], in_ap=ertb[:],
                                idxs_ap=di[:, s0 * 8:(s0 + sn) * 8],
                                num_idxs=sn * 128, num_idxs_reg=sn * 128,
                                elem_size=64, queue_num=0)
                        sl = pool.tile([128, tb], f32, tag="sl")
                        nc.sync.dma_start(sl[:], slot_d[:, t0:t0 + tb])
                        B = pool.tile([128, tb, 128], bf16, tag="B")
                        nc.vector.tensor_tensor(
                            out=B[:],
                            in0=iota_sb[:].unsqueeze(1)
                            .broadcast_to([128, tb, 128]),
                            in1=sl[:].unsqueeze(2)
                            .broadcast_to([128, tb, 128]),
                            op=OP.is_equal)
                        ex = pool.tile([128, tb, sw], f32, tag="ex")
                        nc.vector.tensor_add(ex[:], gt[:, :, fw:fw + sw],
                                             erg[:, :, 0:sw])
                        nc.vector.scalar_tensor_tensor(
                            out=ex[:], in0=ex[:], scalar=NEG_SLOPE,
                            in1=ex[:], op0=OP.mult, op1=OP.max)
                        nc.scalar.activation(ex[:], ex[:], AF.Exp)
                        comb = pool.tile([128, tb, nw], bf16, tag="comb")
                        nc.scalar.copy(comb[:, :, fw:fw + sw], ex[:])
                        nc.vector.tensor_mul(
                            comb[:, :, 0:fw].rearrange(
                                "p t (s d) -> p t s d", s=sw),
                            gt[:, :, 0:fw].rearrange(
                                "p t (s d) -> p t s d", s=sw),
                            ex[:].unsqueeze(3)
                            .broadcast_to([128, tb, sw, fw // sw]))
                        for k in range(b * BLK, min((b + 1) * BLK, nchunk)):
                            tl = chunk_tiles[k]
                            ps = pps.tile([128, nw], f32, tag="agg")
                            for j, t in enumerate(tl):
                                nc.tensor.matmul(
                                    ps[:], lhsT=B[:, t - t0, :],
                                    rhs=comb[:, t - t0, :],
                                    start=(j == 0), stop=(j == len(tl) - 1))
                            mrows = min(CH, sh - k * CH)
                            if layer == 1:
                                post_chunk_l1(k, ps, mrows, ppost, pps2)
                            else:
                                post_chunk_l2(k, ps, mrows, ppost)

            edge_phase(1)
            nc.gpsimd.collective_compute(
                "AllGather", OP.bypass, replica_groups=replica,
                ins=[l2shard.opt()], outs=[l2tab.opt()])
            edge_phase(2)

    nc_.compile()
    return nc_


# ------------------------------------------------------------------ driver
def make_in_maps(meta, feat, W1, al1, ar1, b1, W2, al2, ar2, b2):
    sh = meta["sh"]
    alrep = np.tile(al1.reshape(1, -1), (128, 1)).astype(np.float32)
    arrep = np.tile(ar1.reshape(1, -1), (128, 1)).astype(np.float32)
    b1rep = np.tile(b1.reshape(1, -1), (128, 1)).astype(np.float32)
    W2a = np.concatenate(
        [W2, W2 @ al2.reshape(-1, 1), W2 @ ar2.reshape(-1, 1)],
        axis=1).astype(np.float32)
    b2rep = np.tile(b2.reshape(1, -1), (128, 1)).astype(np.float32)
    in_maps = []
    for c in range(NC):
        in_maps.append({
            "featT": np.ascontiguousarray(feat[c * sh:(c + 1) * sh, :].T),
            "W1": W1, "alrep": alrep, "arrep": arrep, "b1rep": b1rep,
            "W2a": W2a, "b2rep": b2rep,
            "gidx": _wrap16(meta["gidx"][c]),
            "didx": _wrap16(meta["didx"][c]),
            "slot": np.ascontiguousarray(
                meta["slot"][c].reshape(-1, 128).T).astype(np.float32),
        })
    return in_maps


class Runner:
    """Builds the SPMD program once; exposes a repeatable timed executor."""

    def __init__(self, meta, f_in):
        self.meta = meta
        self.nc = build_program(meta, f_in, HID, HEADS, N_CLASSES)
        self._fn = None
        self.repeat = 1

    repeat = 1

    def _lower(self):
        import jax
        import numpy as _np
        from jax.sharding import Mesh, PartitionSpec
        from jax.experimental.shard_map import shard_map
        from concourse import mybir
        from concourse.bass2jax import _bass_exec_p, install_neuronx_cc_hook

        install_neuronx_cc_hook()
        nc = self.nc
        in_names, out_names, out_avals, zero_outs = [], [], [], []
        partition_name = (nc.partition_id_tensor.name
                          if nc.partition_id_tensor else None)
        for alloc in nc.m.functions[0].allocations:
            if not isinstance(alloc, mybir.MemoryLocationSet):
                continue
            name = alloc.memorylocations[0].name
            if alloc.kind == "ExternalInput":
                if name != partition_name:
                    in_names.append(name)
            elif alloc.kind == "ExternalOutput":
                shape = tuple(alloc.tensor_shape)
                dtype = mybir.dt.np(alloc.dtype)
                out_names.append(name)
                out_avals.append(jax.core.ShapedArray(shape, dtype))
                zero_outs.append(_np.zeros(shape, dtype))
        n_params = len(in_names)
        n_outs = len(out_avals)
        all_in_names = list(in_names) + list(out_names)
        if partition_name is not None:
            all_in_names.append(partition_name)

        repeat = self.repeat

        def _body(*args):
            ins = list(args[:n_params])
            zouts = list(args[n_params:])
            outs = None
            for _ in range(repeat):
                operands = ins + zouts
                if partition_name is not None:
                    from concourse.bass2jax import partition_id_tensor
                    operands.append(partition_id_tensor())
                outs = _bass_exec_p.bind(
                    *operands, out_avals=tuple(out_avals),
                    in_names=tuple(all_in_names), out_names=tuple(out_names),
                    lowering_input_output_aliases=(),
                    sim_require_finite=True, sim_require_nnan=True, nc=nc)
                zouts = list(outs)   # chain: serialize + reuse as out bufs
            return tuple(outs)

        devices = jax.devices()[:NC]
        mesh = Mesh(_np.asarray(devices), ("core",))
        in_specs = (PartitionSpec("core"),) * (n_params + n_outs)
        out_specs = (PartitionSpec("core"),) * n_outs
        self._fn = jax.jit(
            shard_map(_body, mesh=mesh, in_specs=in_specs,
                      out_specs=out_specs, check_rep=False),
            keep_unused=True)
        self._in_names = in_names
        self._out_names = out_names
        self._out_avals = out_avals
        self._zero_outs = zero_outs
        self._mesh = mesh
        self._in_specs = in_specs

    def prepare(self, in_maps):
        import jax
        import numpy as _np
        from jax.sharding import NamedSharding, PartitionSpec
        if self._fn is None:
            self._lower()
        concat_in = [
            _np.concatenate([in_maps[c][name] for c in range(NC)], axis=0)
            for name in self._in_names]
        concat_zeros = [
            _np.zeros((NC * z.shape[0], *z.shape[1:]), z.dtype)
            for z in self._zero_outs]
        shd = NamedSharding(self._mesh, PartitionSpec("core"))
        self._args = [jax.device_put(a, shd) for a in concat_in + concat_zeros]
        jax.block_until_ready(self._args)

    def run(self):
        import jax
        out = self._fn(*self._args)
        out = jax.block_until_ready(out)
        import numpy as _np
        res = _np.asarray(out[self._out_names.index("out")])
        sh = self._out_avals[self._out_names.index("out")].shape
        return res.reshape(NC, *sh).reshape(NC * sh[0], *sh[1:])


_RUNNER = None


def get_runner(feat, src, dst):
    global _RUNNER
    n, f_in = feat.shape
    meta = host_prep(np.asarray(src, np.int32), np.asarray(dst, np.int32),
                     n_nodes=n)
    _RUNNER = Runner(meta, f_in)
    return _RUNNER


def kernel(feat, src, dst, W1, al1, ar1, b1, W2, al2, ar2, b2):
    feat = np.asarray(feat, dtype=np.float32)
    src = np.asarray(src, dtype=np.int32)
    dst = np.asarray(dst, dtype=np.int32)
    args = [np.asarray(x, np.float32)
            for x in (W1, al1, ar1, b1, W2, al2, ar2, b2)]
    r = _RUNNER if _RUNNER is not None else get_runner(feat, src, dst)
    in_maps = make_in_maps(r.meta, feat, *args)
    r.prepare(in_maps)
    return r.run()


kernel.last_exec_time_ns = None
